# revision 12
# baseline (speedup 1.0000x reference)
"""GAT layer kernel for 8 Trainium2 NeuronCores.

Strategy (row-sharded attention, per the sharding hint):
  - Core c owns query rows [c*1024, (c+1)*1024) of the 8000-node graph
    (1024 = 8*128; core 7's slice is host-padded; keys padded to
    8064 = 63*128).
  - Rank-1 score factorization: e_ij = s_i + t_j, so
    exp(lrelu(e)) = max(exp(s_i)exp(t_j), exp(.2 s_i)exp(.2 t_j))
                  = max(u_i * v_j, p_i * q_j)
    with u,v,p,q host-precomputed O(N) vectors.  This removes the N^2
    exp (ScalarE) and the N^2 add/lrelu passes of the naive scheme.
  - Per 128-key x 1024-query tile (transposed layout, keys on
    partitions):
      t1 = u_bcast * v_j     (VectorE tensor_scalar, 4x mode)
      t2 = p_bcast * q_j     (ScalarE activation-scale / GpSimd / DVE)
      t3 = max(t1, t2)       (tensor_tensor max, group-chunked,
                              split DVE / GpSimd)
      pt = mask * t3         (tensor_tensor mult, group-chunked,
                              split DVE / GpSimd; mask f16 {0,1}
                              loaded raw by HWDGE - no SWDGE cast,
                              which measured 5.6x slower than raw)
  - attn_unnorm @ [H' | 1] accumulates into 8 PSUM banks; column 256
    is the softmax denominator, applied as a reciprocal while draining
    (W_b folded into the output add since softmax rows sum to 1).
"""
import os
import sys

sys.path.insert(0, "/opt/trn_rl_repo")

import numpy as np
import ml_dtypes

N, F = 8000, 256
NP = 8064          # padded key count (63 * 128)
W = 1024           # query rows per core (8 * 128; last core partly padding)
NJT = NP // 128    # 63 key tiles
GRP = 9            # key tiles per mask-DMA group
NGRP = NJT // GRP  # 7 groups
MB = W // 8        # 128-row output blocks
NEG_SLOPE = 0.2

# engine assignment knobs (within each 9-tile group)
T2_ACT = int(os.environ.get("GAT_T2_ACT", "6"))      # tiles t < this: t2 on ScalarE
T2_POOL = int(os.environ.get("GAT_T2_POOL", "0"))    # next this many: t2 on GpSimd
MAXG_POOL = int(os.environ.get("GAT_MAXG_POOL", "1"))  # of 9 tile-widths of max on GpSimd
MASKG_POOL = int(os.environ.get("GAT_MASKG_POOL", "1"))  # of 9 tile-widths of mask on GpSimd

_RUNNER = None
_last_in_maps = None


def _build(repeat=1):
    import concourse.bass as bass
    import concourse.tile as tile
    from concourse import bacc, mybir

    f16 = mybir.dt.float16
    f32 = mybir.dt.float32

    nc = bacc.Bacc()
    maskd = nc.dram_tensor("maskg", (NGRP * 128, GRP * W), f16, kind="ExternalInput")
    hpd = nc.dram_tensor("hp", (128, NJT * 257), f16, kind="ExternalInput")
    ubd = nc.dram_tensor("ub", (128, W), f16, kind="ExternalInput")
    pbd = nc.dram_tensor("pb", (128, W), f16, kind="ExternalInput")
    vqd = nc.dram_tensor("vq", (128, 2 * NJT), f32, kind="ExternalInput")
    wbd = nc.dram_tensor("wbt", (128, F), f16, kind="ExternalInput")
    outd = nc.dram_tensor("out", (W, F), f32, kind="ExternalOutput")

    with tile.TileContext(nc) as tc:
        with (
            tc.tile_pool(name="pp", bufs=1) as pp,
            tc.tile_pool(name="att", bufs=2) as ap_,
            tc.tile_pool(name="fin", bufs=2) as fin,
            tc.tile_pool(name="ps", bufs=8, space="PSUM") as psp,
        ):
            for _rep in range(repeat):
                # ---- phase 0: all parameters (H' precomputed on host) ----
                hp = pp.tile([128, NJT * 257], f16, name="hp", tag="hp")
                nc.sync.dma_start(hp[:], hpd[:])
                ub = pp.tile([128, W], f16, name="ub", tag="ub", bufs=2)
                nc.sync.dma_start(ub[:], ubd[:])
                pb = pp.tile([128, W], f16, name="pb", tag="pb", bufs=2)
                nc.sync.dma_start(pb[:], pbd[:])
                vq = pp.tile([128, 2 * NJT], f32, name="vq", tag="vq", bufs=2)
                nc.sync.dma_start(vq[:], vqd[:])
                wb_sb = pp.tile([128, F], f16, name="wb", tag="wb", bufs=2)
                nc.sync.dma_start(wb_sb[:], wbd[:])

                # ---- phase 1: masked attention weights + matmul accumulate ----
                po = [psp.tile([MB, 257], f32, name=f"po{ib}", tag="ps") for ib in range(8)]
                for g in range(NGRP):
                    mkf = ap_.tile([128, GRP * W], f16, name="mkf", tag="mkf", bufs=2)
                    nc.sync.dma_start(mkf[:], maskd[g * 128 : (g + 1) * 128, :])
                    t1g = ap_.tile([128, GRP * W], f16, name="t1g", tag="t1g", bufs=2)
                    t2g = ap_.tile([128, GRP * W], f16, name="t2g", tag="t2g", bufs=2)
                    t3g = ap_.tile([128, GRP * W], f16, name="t3g", tag="t3g", bufs=1)
                    ptg = ap_.tile([128, GRP * W], f16, name="ptg", tag="ptg", bufs=2)
                    # 3-tile sub-chunks: matmuls release every chunk so the
                    # PE never sees a multi-us idle gap (HAM re-throttle)
                    for c0 in range(0, GRP, 3):
                        for t in range(c0, c0 + 3):
                            jt = g * GRP + t
                            nc.vector.tensor_scalar_mul(
                                t1g[:, t * W : (t + 1) * W], ub[:],
                                vq[:, 2 * jt : 2 * jt + 1],
                            )
                            if t < T2_ACT:
                                nc.scalar.activation(
                                    t2g[:, t * W : (t + 1) * W], pb[:],
                                    mybir.ActivationFunctionType.Identity,
                                    bias=0.0, scale=vq[:, 2 * jt + 1 : 2 * jt + 2],
                                )
                            else:
                                nc.vector.tensor_scalar_mul(
                                    t2g[:, t * W : (t + 1) * W], pb[:],
                                    vq[:, 2 * jt + 1 : 2 * jt + 2],
                                )
                        lo, hi = c0 * W, (c0 + 3) * W
                        nc.vector.tensor_tensor(
                            t3g[:, lo:hi], t1g[:, lo:hi], t2g[:, lo:hi],
                            mybir.AluOpType.max,
                        )
                        if c0 + 3 >= GRP and MASKG_POOL:
                            cutm = (GRP - MASKG_POOL) * W
                            nc.vector.tensor_tensor(
                                ptg[:, lo:cutm], mkf[:, lo:cutm], t3g[:, lo:cutm],
                                mybir.AluOpType.mult,
                            )
                            nc.gpsimd.tensor_mul(
                                ptg[:, cutm:], mkf[:, cutm:], t3g[:, cutm:]
                            )
                        else:
                            nc.vector.tensor_tensor(
                                ptg[:, lo:hi], mkf[:, lo:hi], t3g[:, lo:hi],
                                mybir.AluOpType.mult,
                            )
                        for t in range(c0, c0 + 3):
                            jt = g * GRP + t
                            for ib in range(8):
                                nc.tensor.matmul(
                                    po[ib][:],
                                    ptg[:, t * W + ib * MB : t * W + (ib + 1) * MB],
                                    hp[:, jt * 257 : (jt + 1) * 257],
                                    start=(jt == 0),
                                    stop=(jt == NJT - 1),
                                )

                # ---- phase 2: normalize + store ----
                for ib in range(8):
                    r = fin.tile([MB, 1], f32, name="rcol", tag="rcol")
                    nc.vector.reciprocal(r[:], po[ib][:, 256:257])
                    ob = fin.tile([MB, F], f32, name="ob", tag="ob")
                    nc.vector.scalar_tensor_tensor(
                        ob[:], po[ib][:, 0:F], r[:], wb_sb[0:MB, :],
                        mybir.AluOpType.mult, mybir.AluOpType.add,
                    )
                    nc.sync.dma_start(outd[ib * MB : (ib + 1) * MB, :], ob[:])

    nc.compile()
    return _make_runner(nc, 8)


def _make_runner(nc, n_cores):
    """Compile-once sharded PJRT runner for the 8-core axon path."""
    import time
    import jax
    from jax.sharding import Mesh, PartitionSpec
    from jax.experimental.shard_map import shard_map
    from concourse import mybir
    from concourse.bass2jax import (
        _bass_exec_p,
        install_neuronx_cc_hook,
        partition_id_tensor,
    )

    install_neuronx_cc_hook()
    partition_name = nc.partition_id_tensor.name if nc.partition_id_tensor else None
    in_names, out_names, out_avals, zero_outs = [], [], [], []
    for alloc in nc.m.functions[0].allocations:
        if not isinstance(alloc, mybir.MemoryLocationSet):
            continue
        name = alloc.memorylocations[0].name
        if alloc.kind == "ExternalInput":
            if name != partition_name:
                in_names.append(name)
        elif alloc.kind == "ExternalOutput":
            out_names.append(name)
            shape = tuple(alloc.tensor_shape)
            dtype = mybir.dt.np(alloc.dtype)
            out_avals.append(jax.core.ShapedArray(shape, dtype))
            zero_outs.append(np.zeros(shape, dtype))
    n_params = len(in_names)
    all_in = in_names + out_names + ([partition_name] if partition_name else [])

    def _body(*args):
        operands = list(args)
        if partition_name is not None:
            operands.append(partition_id_tensor())
        return tuple(
            _bass_exec_p.bind(
                *operands,
                out_avals=tuple(out_avals),
                in_names=tuple(all_in),
                out_names=tuple(out_names),
                lowering_input_output_aliases=(),
                sim_require_finite=True,
                sim_require_nnan=True,
                nc=nc,
            )
        )

    devices = jax.devices()[:n_cores]
    mesh = Mesh(np.asarray(devices), ("core",))
    fn = jax.jit(
        shard_map(
            _body,
            mesh=mesh,
            in_specs=(PartitionSpec("core"),) * (n_params + len(out_names)),
            out_specs=(PartitionSpec("core"),) * len(out_names),
            check_rep=False,
        ),
        keep_unused=True,
    )

    def run(in_maps, iters=0):
        per_core = [[np.asarray(m[n]) for n in in_names] for m in in_maps]
        concat_in = [
            np.concatenate([per_core[c][i] for c in range(n_cores)], axis=0)
            for i in range(n_params)
        ]
        concat_zeros = [
            np.zeros((n_cores * z.shape[0], *z.shape[1:]), z.dtype) for z in zero_outs
        ]
        args = [jax.device_put(a) for a in concat_in + concat_zeros]
        out = fn(*args)
        jax.block_until_ready(out)
        times = []
        for _ in range(iters):
            t0 = time.perf_counter()
            out = fn(*args)
            jax.block_until_ready(out)
            times.append(time.perf_counter() - t0)
        results = [
            {
                name: np.asarray(out[i]).reshape(n_cores, *out_avals[i].shape)[c]
                for i, name in enumerate(out_names)
            }
            for c in range(n_cores)
        ]
        return results, (min(times) if times else None)

    return run


def kernel(node_embeddings, adj_matrix, W_w, W_b, a_src, a_dst, a_b):
    global _RUNNER, _last_in_maps
    if _RUNNER is None:
        _RUNNER = _build()

    X = np.asarray(node_embeddings, np.float32)
    adj = np.asarray(adj_matrix, np.int32)
    W_w = np.asarray(W_w, np.float32)
    W_b = np.asarray(W_b, np.float32)
    a_src = np.asarray(a_src, np.float32)
    a_dst = np.asarray(a_dst, np.float32)
    a_b = float(np.asarray(a_b))

    H0 = X @ W_w.T                            # [N, F] bias-free (W_b folded
    H = H0 + W_b                              # into the final add: softmax
    s = H @ a_src + a_b                       # rows sum to 1)
    t = H @ a_dst                             # key-side score part
    u = np.exp(s)
    pvec = np.exp(NEG_SLOPE * s)
    v = np.zeros(NP, np.float32)
    q = np.zeros(NP, np.float32)
    v[:N] = np.exp(t)
    q[:N] = np.exp(NEG_SLOPE * t)

    # H' tiles, key-transposed with the ones (denominator) column baked in
    Hp = np.zeros((NP, F), np.float32)
    Hp[:N] = H0
    hph = np.ones((128, NJT, 257), np.float16)
    hph[:, :, :256] = Hp.reshape(NJT, 128, F).transpose(1, 0, 2)
    hph = np.ascontiguousarray(hph.reshape(128, NJT * 257))

    v_r = np.ascontiguousarray(v.reshape(NJT, 128).T)  # [128, NJT]
    q_r = np.ascontiguousarray(q.reshape(NJT, 128).T)
    vq = np.empty((128, 2 * NJT), np.float32)
    vq[:, 0::2] = v_r
    vq[:, 1::2] = q_r

    wbt = np.ascontiguousarray(np.tile(W_b, (128, 1))).astype(np.float16)

    in_maps = []
    for c in range(8):
        r0 = c * W
        rows = min(W, N - r0)
        mt = np.zeros((NP, W), np.float16)
        mt[:N, :rows] = adj[r0 : r0 + rows, :].T
        if rows < W:
            mt[0, rows:] = 1  # fake edge keeps padded-query denominators finite
        mg = np.ascontiguousarray(
            mt.reshape(NGRP, GRP, 128, W).transpose(0, 2, 1, 3)
        ).reshape(NGRP * 128, GRP * W)
        uc = np.ones(W, np.float32)
        pc = np.ones(W, np.float32)
        uc[:rows] = u[r0 : r0 + rows]
        pc[:rows] = pvec[r0 : r0 + rows]
        ub = np.ascontiguousarray(
            np.broadcast_to(uc.astype(np.float16), (128, W))
        )
        pbb = np.ascontiguousarray(
            np.broadcast_to(pc.astype(np.float16), (128, W))
        )
        in_maps.append(
            {"maskg": mg, "hp": hph, "ub": ub, "pb": pbb, "vq": vq, "wbt": wbt}
        )

    _last_in_maps = in_maps
    results, _ = _RUNNER(in_maps, iters=0)
    out = np.empty((N, F), np.float32)
    for c in range(8):
        r0 = c * W
        rows = min(W, N - r0)
        out[r0 : r0 + rows] = results[c]["out"][:rows]
    return out


# revision 15
# speedup vs baseline: 1.1156x; 1.1156x over previous
"""GAT layer kernel for 8 Trainium2 NeuronCores.

Strategy (row-sharded attention, per the sharding hint):
  - Core c owns query rows [c*1024, (c+1)*1024) of the 8000-node graph
    (1024 = 8*128; core 7's slice is host-padded; keys padded to
    8064 = 63*128).
  - Rank-1 score factorization: e_ij = s_i + t_j, so
    exp(lrelu(e)) = max(exp(s_i)exp(t_j), exp(.2 s_i)exp(.2 t_j))
                  = max(u_i * v_j, p_i * q_j)
    with u,v,p,q host-precomputed O(N) vectors.  This removes the N^2
    exp (ScalarE) and the N^2 add/lrelu passes of the naive scheme.
  - Per 128-key x 1024-query tile (transposed layout, keys on
    partitions):
      t1 = u_bcast * v_j     (VectorE tensor_scalar, 4x mode)
      t2 = p_bcast * q_j     (ScalarE activation-scale / GpSimd / DVE)
      t3 = max(t1, t2)       (tensor_tensor max, group-chunked,
                              split DVE / GpSimd)
      pt = mask * t3         (tensor_tensor mult, group-chunked,
                              split DVE / GpSimd; mask f16 {0,1}
                              loaded raw by HWDGE - no SWDGE cast,
                              which measured 5.6x slower than raw)
  - attn_unnorm @ [H' | 1] accumulates into 8 PSUM banks; column 256
    is the softmax denominator, applied as a reciprocal while draining
    (W_b folded into the output add since softmax rows sum to 1).
"""
import os
import sys

sys.path.insert(0, "/opt/trn_rl_repo")

import numpy as np
import ml_dtypes

N, F = 8000, 256
NP = 8064          # padded key count (63 * 128)
W = 1024           # query rows per core (8 * 128; last core partly padding)
NJT = NP // 128    # 63 key tiles
GRP = 9            # key tiles per mask-DMA group
NGRP = NJT // GRP  # 7 groups
MB = W // 8        # 128-row output blocks
NEG_SLOPE = 0.2

# engine assignment knobs
T2_ACT = int(os.environ.get("GAT_T2_ACT", "9"))      # tiles t < this (of 9): t2 on ScalarE
# columns of each 3-tile chunk's mask-mult done by GpSimd (rest on VectorE)
MASK_POOL_COLS = int(os.environ.get("GAT_MASK_POOL_COLS", "1280"))

_RUNNER = None
_last_in_maps = None


def _build(repeat=1):
    import concourse.bass as bass
    import concourse.tile as tile
    from concourse import bacc, mybir

    f16 = mybir.dt.float16
    f32 = mybir.dt.float32

    nc = bacc.Bacc()
    maskd = nc.dram_tensor("maskg", (NGRP * 128, GRP * W), f16, kind="ExternalInput")
    hpd = nc.dram_tensor("hp", (128, NJT * 257), f16, kind="ExternalInput")
    ubd = nc.dram_tensor("ub", (128, W), f16, kind="ExternalInput")
    pbd = nc.dram_tensor("pb", (128, W), f16, kind="ExternalInput")
    vqd = nc.dram_tensor("vq", (128, 2 * NJT), f32, kind="ExternalInput")
    wbd = nc.dram_tensor("wbt", (128, F), f16, kind="ExternalInput")
    outd = nc.dram_tensor("out", (W, F), f32, kind="ExternalOutput")

    with tile.TileContext(nc) as tc:
        with (
            tc.tile_pool(name="pp", bufs=1) as pp,
            tc.tile_pool(name="att", bufs=2) as ap_,
            tc.tile_pool(name="fin", bufs=2) as fin,
            tc.tile_pool(name="ps", bufs=8, space="PSUM") as psp,
        ):
            for _rep in range(repeat):
                # ---- phase 0: all parameters (H' precomputed on host) ----
                hp = pp.tile([128, NJT * 257], f16, name="hp", tag="hp")
                nc.sync.dma_start(hp[:], hpd[:])
                ub = pp.tile([128, W], f16, name="ub", tag="ub", bufs=2)
                nc.sync.dma_start(ub[:], ubd[:])
                pb = pp.tile([128, W], f16, name="pb", tag="pb", bufs=2)
                nc.sync.dma_start(pb[:], pbd[:])
                vq = pp.tile([128, 2 * NJT], f32, name="vq", tag="vq", bufs=2)
                nc.sync.dma_start(vq[:], vqd[:])
                wb_sb = pp.tile([128, F], f16, name="wb", tag="wb", bufs=2)
                nc.sync.dma_start(wb_sb[:], wbd[:])

                # ---- phase 1: masked attention weights + matmul accumulate ----
                po = [psp.tile([MB, 257], f32, name=f"po{ib}", tag="ps") for ib in range(8)]
                for g in range(NGRP):
                    mkf = ap_.tile([128, GRP * W], f16, name="mkf", tag="mkf", bufs=2)
                    nc.sync.dma_start(mkf[:], maskd[g * 128 : (g + 1) * 128, :])
                    t1g = ap_.tile([128, GRP * W], f16, name="t1g", tag="t1g", bufs=2)
                    t2g = ap_.tile([128, GRP * W], f16, name="t2g", tag="t2g", bufs=2)
                    t3g = ap_.tile([128, GRP * W], f16, name="t3g", tag="t3g", bufs=1)
                    ptg = ap_.tile([128, GRP * W], f16, name="ptg", tag="ptg", bufs=2)
                    # 3-tile sub-chunks: matmuls release every chunk so the
                    # PE never sees a multi-us idle gap (HAM re-throttle)
                    for c0 in range(0, GRP, 3):
                        for t in range(c0, c0 + 3):
                            jt = g * GRP + t
                            nc.vector.tensor_scalar_mul(
                                t1g[:, t * W : (t + 1) * W], ub[:],
                                vq[:, 2 * jt : 2 * jt + 1],
                            )
                            if t < T2_ACT:
                                nc.scalar.activation(
                                    t2g[:, t * W : (t + 1) * W], pb[:],
                                    mybir.ActivationFunctionType.Identity,
                                    bias=0.0, scale=vq[:, 2 * jt + 1 : 2 * jt + 2],
                                )
                            else:
                                nc.vector.tensor_scalar_mul(
                                    t2g[:, t * W : (t + 1) * W], pb[:],
                                    vq[:, 2 * jt + 1 : 2 * jt + 2],
                                )
                        lo, hi = c0 * W, (c0 + 3) * W
                        nc.vector.tensor_tensor(
                            t3g[:, lo:hi], t1g[:, lo:hi], t2g[:, lo:hi],
                            mybir.AluOpType.max,
                        )
                        cutm = hi - MASK_POOL_COLS
                        nc.vector.tensor_tensor(
                            ptg[:, lo:cutm], mkf[:, lo:cutm], t3g[:, lo:cutm],
                            mybir.AluOpType.mult,
                        )
                        if MASK_POOL_COLS:
                            nc.gpsimd.tensor_mul(
                                ptg[:, cutm:hi], mkf[:, cutm:hi], t3g[:, cutm:hi]
                            )
                        for t in range(c0, c0 + 3):
                            jt = g * GRP + t
                            for ib in range(8):
                                for h in range(2):
                                    # column-tiled 128x64: the two col-tiles
                                    # ping-pong LDW under each other's stream
                                    nc.tensor.matmul(
                                        po[ib][h * 64 : (h + 1) * 64, :],
                                        ptg[:, t * W + ib * MB + h * 64 : t * W + ib * MB + (h + 1) * 64],
                                        hp[:, jt * 257 : (jt + 1) * 257],
                                        start=(jt == 0),
                                        stop=(jt == NJT - 1),
                                        tile_position=(0, h * 64),
                                    )

                # ---- phase 2: normalize + store ----
                for ib in range(8):
                    r = fin.tile([MB, 1], f32, name="rcol", tag="rcol")
                    nc.vector.reciprocal(r[:], po[ib][:, 256:257])
                    ob = fin.tile([MB, F], f32, name="ob", tag="ob")
                    nc.vector.scalar_tensor_tensor(
                        ob[:], po[ib][:, 0:F], r[:], wb_sb[0:MB, :],
                        mybir.AluOpType.mult, mybir.AluOpType.add,
                    )
                    nc.sync.dma_start(outd[ib * MB : (ib + 1) * MB, :], ob[:])

    nc.compile()
    return _make_runner(nc, 8)


def _make_runner(nc, n_cores):
    """Compile-once sharded PJRT runner for the 8-core axon path."""
    import time
    import jax
    from jax.sharding import Mesh, PartitionSpec
    from jax.experimental.shard_map import shard_map
    from concourse import mybir
    from concourse.bass2jax import (
        _bass_exec_p,
        install_neuronx_cc_hook,
        partition_id_tensor,
    )

    install_neuronx_cc_hook()
    partition_name = nc.partition_id_tensor.name if nc.partition_id_tensor else None
    in_names, out_names, out_avals, zero_outs = [], [], [], []
    for alloc in nc.m.functions[0].allocations:
        if not isinstance(alloc, mybir.MemoryLocationSet):
            continue
        name = alloc.memorylocations[0].name
        if alloc.kind == "ExternalInput":
            if name != partition_name:
                in_names.append(name)
        elif alloc.kind == "ExternalOutput":
            out_names.append(name)
            shape = tuple(alloc.tensor_shape)
            dtype = mybir.dt.np(alloc.dtype)
            out_avals.append(jax.core.ShapedArray(shape, dtype))
            zero_outs.append(np.zeros(shape, dtype))
    n_params = len(in_names)
    all_in = in_names + out_names + ([partition_name] if partition_name else [])

    def _body(*args):
        operands = list(args)
        if partition_name is not None:
            operands.append(partition_id_tensor())
        return tuple(
            _bass_exec_p.bind(
                *operands,
                out_avals=tuple(out_avals),
                in_names=tuple(all_in),
                out_names=tuple(out_names),
                lowering_input_output_aliases=(),
                sim_require_finite=True,
                sim_require_nnan=True,
                nc=nc,
            )
        )

    devices = jax.devices()[:n_cores]
    mesh = Mesh(np.asarray(devices), ("core",))
    fn = jax.jit(
        shard_map(
            _body,
            mesh=mesh,
            in_specs=(PartitionSpec("core"),) * (n_params + len(out_names)),
            out_specs=(PartitionSpec("core"),) * len(out_names),
            check_rep=False,
        ),
        keep_unused=True,
    )

    def run(in_maps, iters=0):
        per_core = [[np.asarray(m[n]) for n in in_names] for m in in_maps]
        concat_in = [
            np.concatenate([per_core[c][i] for c in range(n_cores)], axis=0)
            for i in range(n_params)
        ]
        concat_zeros = [
            np.zeros((n_cores * z.shape[0], *z.shape[1:]), z.dtype) for z in zero_outs
        ]
        args = [jax.device_put(a) for a in concat_in + concat_zeros]
        out = fn(*args)
        jax.block_until_ready(out)
        times = []
        for _ in range(iters):
            t0 = time.perf_counter()
            out = fn(*args)
            jax.block_until_ready(out)
            times.append(time.perf_counter() - t0)
        results = [
            {
                name: np.asarray(out[i]).reshape(n_cores, *out_avals[i].shape)[c]
                for i, name in enumerate(out_names)
            }
            for c in range(n_cores)
        ]
        return results, (min(times) if times else None)

    return run


def kernel(node_embeddings, adj_matrix, W_w, W_b, a_src, a_dst, a_b):
    global _RUNNER, _last_in_maps
    if _RUNNER is None:
        _RUNNER = _build()

    X = np.asarray(node_embeddings, np.float32)
    adj = np.asarray(adj_matrix, np.int32)
    W_w = np.asarray(W_w, np.float32)
    W_b = np.asarray(W_b, np.float32)
    a_src = np.asarray(a_src, np.float32)
    a_dst = np.asarray(a_dst, np.float32)
    a_b = float(np.asarray(a_b))

    H0 = X @ W_w.T                            # [N, F] bias-free (W_b folded
    H = H0 + W_b                              # into the final add: softmax
    s = H @ a_src + a_b                       # rows sum to 1)
    t = H @ a_dst                             # key-side score part
    u = np.exp(s)
    pvec = np.exp(NEG_SLOPE * s)
    v = np.zeros(NP, np.float32)
    q = np.zeros(NP, np.float32)
    v[:N] = np.exp(t)
    q[:N] = np.exp(NEG_SLOPE * t)

    # H' tiles, key-transposed with the ones (denominator) column baked in
    Hp = np.zeros((NP, F), np.float32)
    Hp[:N] = H0
    hph = np.ones((128, NJT, 257), np.float16)
    hph[:, :, :256] = Hp.reshape(NJT, 128, F).transpose(1, 0, 2)
    hph = np.ascontiguousarray(hph.reshape(128, NJT * 257))

    v_r = np.ascontiguousarray(v.reshape(NJT, 128).T)  # [128, NJT]
    q_r = np.ascontiguousarray(q.reshape(NJT, 128).T)
    vq = np.empty((128, 2 * NJT), np.float32)
    vq[:, 0::2] = v_r
    vq[:, 1::2] = q_r

    wbt = np.ascontiguousarray(np.tile(W_b, (128, 1))).astype(np.float16)

    in_maps = []
    for c in range(8):
        r0 = c * W
        rows = min(W, N - r0)
        mt = np.zeros((NP, W), np.float16)
        mt[:N, :rows] = adj[r0 : r0 + rows, :].T
        if rows < W:
            mt[0, rows:] = 1  # fake edge keeps padded-query denominators finite
        mg = np.ascontiguousarray(
            mt.reshape(NGRP, GRP, 128, W).transpose(0, 2, 1, 3)
        ).reshape(NGRP * 128, GRP * W)
        uc = np.ones(W, np.float32)
        pc = np.ones(W, np.float32)
        uc[:rows] = u[r0 : r0 + rows]
        pc[:rows] = pvec[r0 : r0 + rows]
        ub = np.ascontiguousarray(
            np.broadcast_to(uc.astype(np.float16), (128, W))
        )
        pbb = np.ascontiguousarray(
            np.broadcast_to(pc.astype(np.float16), (128, W))
        )
        in_maps.append(
            {"maskg": mg, "hp": hph, "ub": ub, "pb": pbb, "vq": vq, "wbt": wbt}
        )

    _last_in_maps = in_maps
    results, _ = _RUNNER(in_maps, iters=0)
    out = np.empty((N, F), np.float32)
    for c in range(8):
        r0 = c * W
        rows = min(W, N - r0)
        out[r0 : r0 + rows] = results[c]["out"][:rows]
    return out
